# revision 1
# baseline (speedup 1.0000x reference)
# MoE kernel for Trainium2 (8 NeuronCores, expert-parallel).
#
# Strategy:
#  - Host: gate logits = x @ gate_w, top-2 + softmax, gather tokens per expert
#    (the "all-to-all by routed expert" from the sharding hint, done host-side
#    since we hold full inputs), pad each expert's token set to a common
#    capacity C (= max expert load, rounded to 128).
#  - Device (core e = expert e): h = gelu(xg^T-major GEMM w1) ; y = h GEMM w2.
#    Both GEMMs in bf16 on the PE array (1 cycle/row), fp32 PSUM accumulate.
#    Token dim rides the matmul free axis; D/dff ride partitions. Inputs are
#    pre-blocked on the host so every DMA chunk is contiguous in DRAM.
#  - Host: scatter-add wts * (y + b2[e]) back into the output.
import math
from contextlib import ExitStack

import ml_dtypes
import numpy as np

import concourse.bass as bass
import concourse.mybir as mybir
import concourse.tile as tile
from concourse.bass_utils import run_bass_kernel_spmd

D = 1024
DFF = 4096
E = 8
TOP_K = 2
P = 128
KD = D // P      # 8  contraction tiles for GEMM1
NF = DFF // P    # 32 dff tiles (GEMM1 out / GEMM2 contraction)
ND = D // P      # 8  GEMM2 out tiles
T_TILE = 512
FG = 4           # w1 f-chunks grouped per tile for big-descriptor DMA
NG = NF // FG    # 8 such groups

BF16 = mybir.dt.bfloat16
F32 = mybir.dt.float32
NP_BF16 = np.dtype(ml_dtypes.bfloat16)

_neff_cache = {}


def _t_tiles(C):
    out, t0 = [], 0
    while t0 < C:
        tsz = min(T_TILE, C - t0)
        out.append((t0, tsz))
        t0 += tsz
    return out


def _split_multiwait_json(bir_bytes: bytes) -> bytes:
    """The walrus build in this container rejects instructions carrying more
    than one sync wait (or update). Split extras onto adjacent single-wait
    EventSemaphore carriers on the same engine: program order on the engine
    preserves the semantics exactly."""
    import json as _json

    bir = _json.loads(bir_bytes)
    for fn in bir["functions"]:
        for blk in fn["blocks"]:
            insts = blk.get("instructions", [])
            out = []
            for inst in insts:
                si = inst.get("sync_info")
                if si:
                    waits = si.get("on_wait") or []
                    if len(waits) > 1:
                        for i, w in enumerate(waits[:-1]):
                            out.append({
                                "debug": inst.get("debug", 0),
                                "engine": inst["engine"],
                                "ins": [],
                                "name": f"{inst['name']}_w{i}",
                                "opcode": "EventSemaphore",
                                "outs": [],
                                "sync_info": {"on_update": [], "on_wait": [w]},
                            })
                        si["on_wait"] = [waits[-1]]
                out.append(inst)
                if si:
                    ups = si.get("on_update") or []
                    if len(ups) > 1:
                        for i, u in enumerate(ups[1:]):
                            out.append({
                                "debug": inst.get("debug", 0),
                                "engine": inst["engine"],
                                "ins": [],
                                "name": f"{inst['name']}_u{i}",
                                "opcode": "EventSemaphore",
                                "outs": [],
                                "sync_info": {"on_update": [u], "on_wait": []},
                            })
                        si["on_update"] = [ups[0]]
            blk["instructions"] = out
    return _json.dumps(bir).encode()


def _patch_to_json(nc: bass.Bass) -> bass.Bass:
    orig = nc.to_json_bytes
    nc.to_json_bytes = lambda: _split_multiwait_json(orig())
    return nc


def _build_bass(C: int) -> bass.Bass:
    """One expert's MLP in transposed layouts (token dim = free axis).

    DRAM input layouts (pre-blocked on host so each partition's span is
    contiguous and large; DMAs are band-split across partitions onto
    parallel HW-DGE queues):
      xs : [n_t * P, KD * T_TILE] bf16; row ti*P+p holds x[kd, col] (8KB)
      w1x: [NG * P, KD * FG * P] bf16; row g*P+p holds w1 f-group g (8KB)
      w2 : [DFF, D] bf16 (natural layout; block f at rows f*P)
      b1 : [DFF] f32
    Output:
      y  : [ND * n_t * P, T_TILE] f32, block (dd, ti) at rows (dd*n_t+ti)*P
    """
    nc = bass.Bass()
    tt = _t_tiles(C)
    n_t = len(tt)
    xs_h = nc.dram_tensor("xs", [n_t * P, KD * T_TILE], BF16, kind="ExternalInput")
    w1_h = nc.dram_tensor("w1", [NG * P, KD * FG * P], BF16, kind="ExternalInput")
    b1_h = nc.dram_tensor("b1", [DFF], F32, kind="ExternalInput")
    w2_h = nc.dram_tensor("w2", [DFF, D], BF16, kind="ExternalInput")
    y_h = nc.dram_tensor("y", [ND * n_t * P, T_TILE], F32, kind="ExternalOutput")

    with ExitStack() as ctx:
        tc = ctx.enter_context(tile.TileContext(nc))
        wpool = ctx.enter_context(tc.tile_pool(name="w", bufs=1))
        xpool = ctx.enter_context(tc.tile_pool(name="x", bufs=1))
        hpool = ctx.enter_context(tc.tile_pool(name="h", bufs=1))
        bpool = ctx.enter_context(tc.tile_pool(name="b", bufs=1))
        ypool = ctx.enter_context(tc.tile_pool(name="y", bufs=3))
        ps1 = ctx.enter_context(tc.tile_pool(name="ps1", bufs=3, space="PSUM"))
        ps2 = ctx.enter_context(tc.tile_pool(name="ps2", bufs=3, space="PSUM"))

        # DMA order: t-tile 0 activations, then w1 (needed by GEMM1), then the
        # rest of the activations, then w2 (needed ~55us in by GEMM2). The
        # critical startup tensors are band-split across partitions so several
        # HW-DGE queues deliver one tile in parallel with 8KB descriptors.
        BANDS = 8
        BP = P // BANDS
        x_t = [None] * n_t
        for ti, (t0, tsz) in enumerate(tt):
            t = xpool.tile([P, KD, T_TILE], BF16, tag=f"x{ti}", name=f"x{ti}")
            for b in range(BANDS):
                r0 = ti * P + b * BP
                nc.sync.dma_start(
                    t[b * BP:(b + 1) * BP, :, :],
                    xs_h[r0:r0 + BP, :].rearrange("p (kd c) -> p kd c", kd=KD),
                )
            x_t[ti] = t
            if ti == 0:
                w1_t = []
                for g in range(NG):
                    t = wpool.tile([P, KD, FG * P], BF16, tag=f"w1_{g}", name=f"w1_{g}")
                    for b in range(4):
                        r0 = g * P + b * 32
                        nc.sync.dma_start(
                            t[b * 32:(b + 1) * 32, :, :],
                            w1_h[r0:r0 + 32, :].rearrange(
                                "p (kd m) -> p kd m", kd=KD),
                        )
                    w1_t.append(t)
        w2_t = []
        for f in range(NF):
            t = wpool.tile([P, D], BF16, tag=f"w2_{f}", name=f"w2_{f}")
            nc.sync.dma_start(t[:], w2_h[f * P:(f + 1) * P, :])
            w2_t.append(t)
        b1_raw = bpool.tile([P, NF], F32)
        nc.gpsimd.dma_start(b1_raw[:], b1_h[:].rearrange("(f p) -> p f", p=P))
        # Funnel b1 through an ACT-engine copy: downstream gelus then reach it
        # via same-engine program order instead of an extra semaphore wait.
        b1_t = bpool.tile([P, NF], F32)
        nc.scalar.copy(b1_t[:], b1_raw[:])

        gelu = mybir.ActivationFunctionType.Gelu
        for ti, (t0, tsz) in enumerate(tt):
            h_t = [hpool.tile([P, T_TILE], BF16, tag=f"h{f}", name=f"h{f}")
                   for f in range(NF)]
            for f in range(NF):
                pt = ps1.tile([P, T_TILE], F32, tag="ps1", name="pt1")
                for k in range(KD):
                    nc.tensor.matmul(
                        pt[:, :tsz],
                        w1_t[f // FG][:, k, (f % FG) * P:(f % FG + 1) * P],
                        x_t[ti][:, k, :tsz],
                        start=(k == 0),
                        stop=(k == KD - 1),
                    )
                nc.scalar.activation(
                    h_t[f][:, :tsz], pt[:, :tsz], gelu, bias=b1_t[:, f:f + 1]
                )
            for dd in range(ND):
                pt2 = ps2.tile([P, T_TILE], F32, tag="ps2", name="pt2")
                for f in range(NF):
                    nc.tensor.matmul(
                        pt2[:, :tsz],
                        w2_t[f][:, dd * P:(dd + 1) * P],
                        h_t[f][:, :tsz],
                        start=(f == 0),
                        stop=(f == NF - 1),
                    )
                y_t = ypool.tile([P, T_TILE], F32, tag="y", name="yt")
                nc.vector.tensor_copy(y_t[:, :tsz], pt2[:, :tsz])
                r0 = (dd * n_t + ti) * P
                nc.sync.dma_start(y_h[r0:r0 + P, :tsz], y_t[:, :tsz])
    return _patch_to_json(nc)


def _route(xf: np.ndarray, gate_w: np.ndarray):
    """Top-2 gating identical to the reference (argmax ties -> lower index)."""
    N = xf.shape[0]
    logits = xf @ gate_w  # (N, E) f32
    rows = np.arange(N)
    i1 = logits.argmax(1)
    v1 = logits[rows, i1]
    masked = logits.copy()
    masked[rows, i1] = -np.inf
    i2 = masked.argmax(1)
    v2 = masked[rows, i2]
    # softmax over the two selected logits (v1 >= v2)
    e = np.exp((v2 - v1).astype(np.float32))
    wt1 = (1.0 / (1.0 + e)).astype(np.float32)
    wt2 = (e / (1.0 + e)).astype(np.float32)
    idx_e, wts_e = [], []
    for ex in range(E):
        s1 = np.nonzero(i1 == ex)[0]
        s2 = np.nonzero(i2 == ex)[0]
        idx_e.append(np.concatenate([s1, s2]))
        wts_e.append(np.concatenate([wt1[s1], wt2[s2]]).astype(np.float32))
    return idx_e, wts_e


def kernel(x, gate_w, w1, b1, w2, b2, _trace=False):
    B, T, D_ = x.shape
    N = B * T
    xf = np.ascontiguousarray(x.reshape(N, D_).astype(np.float32))
    idx_e, wts_e = _route(xf, gate_w.astype(np.float32))
    cnts = [len(i) for i in idx_e]
    C = max(P, int(math.ceil(max(cnts) / P)) * P)
    tt = _t_tiles(C)
    n_t = len(tt)
    C_pad = n_t * T_TILE

    if C in _neff_cache:
        nc = _neff_cache[C]
    else:
        nc = _build_bass(C)
        _neff_cache[C] = nc

    in_maps = []
    for ex in range(E):
        cnt = cnts[ex]
        xg = np.zeros((C_pad, D), np.float32)
        if cnt:
            xg[:cnt] = xf[idx_e[ex]]
        # xs[ti*P + p, kd*T_TILE + col] = xgT[kd*P+p, ti*T_TILE+col]
        xs = (
            xg.T.reshape(KD, P, n_t, T_TILE)
            .transpose(2, 1, 0, 3)
            .reshape(n_t * P, KD * T_TILE)
        )
        # w1x[g*P + p, kd*FG*P + j*P + m] = w1[kd*P+p, (g*FG+j)*P + m]
        w1x = (
            w1[ex]
            .reshape(KD, P, NG, FG * P)
            .transpose(2, 1, 0, 3)
            .reshape(NG * P, KD * FG * P)
        )
        in_maps.append({
            "xs": np.ascontiguousarray(xs).astype(NP_BF16),
            "w1": np.ascontiguousarray(w1x).astype(NP_BF16),
            "b1": np.ascontiguousarray(b1[ex]).astype(np.float32),
            "w2": np.ascontiguousarray(w2[ex]).astype(NP_BF16),
        })

    res = run_bass_kernel_spmd(nc, in_maps, core_ids=list(range(E)), trace=_trace)
    if _trace:
        print(f"HW exec time: {res.exec_time_ns} ns")

    out = np.zeros((N, D), np.float32)
    for ex in range(E):
        cnt = cnts[ex]
        if not cnt:
            continue
        yb = res.results[ex]["y"]  # [ND*n_t*P, T_TILE] f32
        yt = (
            yb.reshape(ND, n_t, P, T_TILE)
            .transpose(0, 2, 1, 3)
            .reshape(D, C_pad)
        )
        yv = yt[:, :cnt].T + b2[ex][None, :].astype(np.float32)
        out[idx_e[ex]] += wts_e[ex][:, None] * yv
    return out.reshape(B, T, D_)

